# revision 1
# baseline (speedup 1.0000x reference)
"""DGCNN forward kernel for Trainium2, data-parallel over 8 NeuronCores.

Structure of the problem (shapes hardcoded from the task spec):
  x          [1_000_000, 64]  fp32   node features, 10_000 graphs x 100 nodes
  batch      arange(N)//100          (graphs are contiguous 100-node blocks)
  4-layer MLP 64->64->64->64->34 with ReLU
  mean-pool over the FIRST 30 nodes of each graph  -> [10_000, 34]
  conv1d(1->16,k=5) + ReLU -> maxpool(2) -> conv1d(16->32,k=5) + ReLU
  flatten -> linear(352->2)

Key optimizations:
  * only the first 30 of every 100 nodes contribute to the output -> host
    slices x down to 30% before shipping to the device.
  * two node-chunks are packed into the 128 SBUF partitions with
    block-diagonal MLP weights (K=64 -> K=128), doubling PE and ACT/DVE
    efficiency.
  * matmuls run as float32r (TF32-like) - 1 cycle/row at N>=256.
  * mean division (1/30) is folded into the conv1 weights; conv1+maxpool
    use an even/odd output split so maxpool is a plain tensor_tensor(max),
    and conv1's ReLU commutes with the max so it is applied once after.
  * conv1/conv2/fc are expressed as banded matmuls over a transposed
    [feature, graph] layout built on host.
"""

import json

import numpy as np

# ---------------------------------------------------------------- constants
N = 1_000_000
G = 10_000
NODES_PER_G = 100
K = 30
F = 64
NCORE = 8
G_CORE = G // NCORE          # 1250 graphs per core
G_HALF = G_CORE // 2         # 625 graphs per packed half
G_HALF_PAD = 640             # padded to 10 tiles of 64 graphs
TILE_G = 64                  # graphs per half per super-tile
N_TILES = G_HALF_PAD // TILE_G
N_CHUNKS = 4                 # 512-col chunks per super-tile
CHUNK_COLS = 512
DATA_COLS = 480              # 16 graphs * 30 nodes
TILE_COLS = N_CHUNKS * CHUNK_COLS   # 2048
HEAD_COLS = 2 * G_HALF_PAD   # 1280 pooled columns (A-half then B-half)

# const tensor column offsets (all fp32, [128, NCOL_PAD])
OFF_W1, OFF_W2, OFF_W3 = 0, 128, 256
OFF_W4 = 384                      # 68 cols
OFF_CE1, OFF_CO1 = 452, 580       # conv1 lhsT, u=0..7  (t even / odd)
OFF_CEM, OFF_COM = 708, 836       # conv1 lhsT, u=4..11
OFF_CE2, OFF_CO2 = 964, 1076      # conv1 lhsT, u=8..14 (112 cols)
OFF_C20, OFF_C21 = 1188, 1316     # conv2 lhsT t=0..3 / 4..7 (128 cols)
OFF_C22 = 1444                    # conv2 lhsT t=8..10 ([112, 96])
OFF_WO0, OFF_WO1, OFF_WO2 = 1540, 1542, 1544
NCOL_PAD = 1568
# fp32 bias tensor columns ([128, 8])
FB_B1, FB_B2, FB_B3, FB_B4, FB_C1B, FB_C2B, FB_BO = 0, 1, 2, 3, 4, 5, 6


# ------------------------------------------------- walrus sync-wait workaround
def _split_sync_waits(bir: dict) -> dict:
    """The walrus build in this container accepts at most ONE sync-wait per
    instruction.  Hoist extra waits onto same-engine EventSemaphore carriers
    (the exact shape wait_ge() emits) inserted right before the instruction;
    engines dispatch in order so semantics are unchanged."""
    for fn in bir.get("functions", []):
        for bb in fn.get("blocks", []):
            out = []
            for inst in bb.get("instructions", []):
                si = inst.get("sync_info") or {}
                ow = si.get("on_wait") or []
                if len(ow) > 1:
                    for k, w in enumerate(ow[:-1]):
                        out.append(
                            {
                                "debug": inst.get("debug"),
                                "engine": inst["engine"],
                                "ins": [],
                                "name": f"{inst['name']}_hw{k}",
                                "opcode": "EventSemaphore",
                                "outs": [],
                                "sync_info": {"on_update": [], "on_wait": [w]},
                            }
                        )
                    si = dict(si)
                    si["on_wait"] = [ow[-1]]
                    inst = dict(inst)
                    inst["sync_info"] = si
                out.append(inst)
            bb["instructions"] = out
    return bir


_patch_installed = False


def _install_bir_patch():
    global _patch_installed
    if _patch_installed:
        return
    import concourse.bass as bass

    orig = bass.Bass.to_json_bytes
    if getattr(bass.Bass, "_ant_sync_wait_patch", False):
        _patch_installed = True
        return

    def patched(self) -> bytes:
        return json.dumps(_split_sync_waits(json.loads(orig(self)))).encode()

    bass.Bass.to_json_bytes = patched
    bass.Bass._ant_sync_wait_patch = True
    _patch_installed = True


# tuning knobs (read by _build_nc; override before first _get_nc call)
TUNE = {
    "xp_bufs": 3,
    "hp_bufs": 2,
    "ps_bufs": 8,
    "skip_head": False,
    "pool_mode": "gpsimd_halve",  # or "dve_full"
    "xdma_split": 4,
    "head_c2relu": "dve",
    "pool_halve2": False,
    "head_interleave": False,
    "pair": "",  # PSUM-pairing of same-engine chunk runs: regresses, keep off
}

# ------------------------------------------------------------- device program
_NC_CACHE = {}


class _nullpool:
    def __enter__(self):
        return None

    def __exit__(self, *a):
        return False


def _build_nc():
    """Build the per-core Bass program (identical on all 8 cores)."""
    _install_bir_patch()
    import concourse.bass as bass
    import concourse.tile as tile
    from concourse import mybir

    f32 = mybir.dt.float32
    f32r = mybir.dt.float32r
    Relu = mybir.ActivationFunctionType.Relu
    ADD = mybir.AluOpType.add
    MAX = mybir.AluOpType.max
    AX = mybir.AxisListType.X

    nc = bass.Bass()
    # all matmul-feeding data is declared float32r (same 4-byte storage as
    # fp32; the PE rounds internally) so the BIR verifier's fp32r-provenance
    # check passes.  Biases live in a separate fp32 tensor.
    xt = nc.dram_tensor("xt", [N_TILES, 128, TILE_COLS], f32r, kind="ExternalInput")
    cst = nc.dram_tensor("cst", [128, NCOL_PAD], f32r, kind="ExternalInput")
    cstf = nc.dram_tensor("cstf", [128, 8], f32, kind="ExternalInput")
    y = nc.dram_tensor("y", [2, HEAD_COLS], f32, kind="ExternalOutput")

    def gapped(ap, parts):
        # [P, 2048] -> [P, 4, 480] view skipping the 32-col pad per chunk
        return ap[0:parts, :].rearrange("p (c s) -> p c s", c=N_CHUNKS)[
            :, :, 0:DATA_COLS
        ]

    with tile.TileContext(nc) as tc:
        with (
            tc.tile_pool(name="persist", bufs=1) as persist,
            tc.tile_pool(name="xp", bufs=TUNE["xp_bufs"]) as xp,
            tc.tile_pool(name="hp", bufs=TUNE["hp_bufs"]) as hp,
            tc.tile_pool(name="hsb", bufs=2) as hsb,
        ):
            # DMA order matters: the model serializes the DMA engines, so
            # load W1 + biases first (tiny), then the first x tiles, then the
            # bulk of the constants.
            cstt = persist.tile([128, NCOL_PAD], f32r)
            cstf_t = persist.tile([128, 8], f32)
            # DMA-FIFO priority order: W1 -> tile-0 x splits (with W2..W4
            # after the first split, biases after the last) -> tile 1 ->
            # head-constant bulk.  Layer k of tile 0 must never wait behind
            # x data it does not need yet.
            wsplit = TUNE.get("w_split", False)  # measured: split regresses
            if wsplit:
                nc.sync.dma_start(out=cstt[:, 0:128], in_=cst[:, 0:128])
            else:
                nc.sync.dma_start(out=cstt[:, 0:452], in_=cst[:, 0:452])
            xt_pre = {}
            nsp0 = TUNE.get("xdma_split0", TUNE.get("xdma_split", 1))
            step0 = TILE_COLS // nsp0
            for i in range(min(TUNE.get("x_prefetch", 2), N_TILES)):
                xt_p = xp.tile([128, TILE_COLS], f32r, name="xt_i")
                for s in range(nsp0):
                    nc.sync.dma_start(
                        out=xt_p[:, s * step0 : (s + 1) * step0],
                        in_=xt[i, :, s * step0 : (s + 1) * step0],
                    )
                    if wsplit and i == 0 and s == 0:
                        nc.sync.dma_start(
                            out=cstt[:, 128:452], in_=cst[:, 128:452]
                        )
                xt_pre[i] = xt_p
                if i == 0:
                    # biases (tiny, needed by tile-0 L1 relu) go after tile
                    # 0's x splits: earlier positions delay the x data more
                    # than the relu gains
                    nc.sync.dma_start(out=cstf_t[:], in_=cstf[:, :])
            # HWDGE (not SWDGE): keeps the head-constant bulk strictly
            # behind the x prefetch in the serialized DMA FIFO - on SWDGE its
            # descriptors are ready early and jump ahead of the x tiles.
            nc.sync.dma_start(
                out=cstt[:, 452:NCOL_PAD], in_=cst[:, 452:NCOL_PAD]
            )
            pooledP = persist.tile([68, 2 * G_HALF_PAD], f32r)  # packed pooled sums
            if TUNE.get("ragged_tail", True):
                # graphs 625:640 are never computed; zero their pooled slots
                # so the head unpack reads defined data (host discards them)
                nc.vector.memset(
                    pooledP[:, G_HALF:G_HALF_PAD].bitcast(f32), 0.0
                )
            pooledU = persist.tile([34, HEAD_COLS], f32r)       # unpacked
            ysb = persist.tile([2, HEAD_COLS], f32)

            layer_cfg = [
                (OFF_W1, 128, FB_B1, 128),
                (OFF_W2, 128, FB_B2, 128),
                (OFF_W3, 128, FB_B3, 128),
                (OFF_W4, 68, FB_B4, 68),
            ]

            # per (layer, chunk) relu engine: 'a' = ACT, 'd' = DVE.
            # 16 chars = 4 layers x 4 chunks.
            emap = TUNE.get("emap", "aaaaddadadadaadd")
            hps_bufs = TUNE.get("hps_bufs", 0)
            ps2_bufs = TUNE.get("ps2_bufs", 0)
            with (
                tc.tile_pool(
                    name="mps",
                    bufs=TUNE["ps_bufs"] - hps_bufs - 2 * ps2_bufs,
                    space="PSUM",
                ) as mps,
                tc.tile_pool(
                    name="mps2", bufs=max(ps2_bufs, 1), space="PSUM"
                ) if ps2_bufs else _nullpool() as mps2,
                tc.tile_pool(
                    name="hps", bufs=max(hps_bufs, 1), space="PSUM"
                ) if hps_bufs else _nullpool() as hps,
            ):
                if hps is None:
                    hps = mps
                if mps2 is None:
                    mps2 = mps
                def _unpack_early():
                    # pooledU cols 0:512 = A graphs 0:512, 512:1024 = B 0:512
                    # (ready after MLP tile 7); host unscrambles columns.
                    nc.gpsimd.dma_start(
                        out=pooledU[:, 0:512], in_=pooledP[0:34, 0:512]
                    )
                    nc.gpsimd.dma_start(
                        out=pooledU[:, 512:1024], in_=pooledP[34:68, 0:512]
                    )

                def _unpack_late():
                    # cols 1024:1152 = A 512:640, 1152:1280 = B 512:640.
                    # HWDGE: these gate the tail head chunk (lower latency
                    # than the SWDGE path).
                    nc.sync.dma_start(
                        out=pooledU[:, 1024:1152], in_=pooledP[0:34, 512:640]
                    )
                    nc.sync.dma_start(
                        out=pooledU[:, 1152:1280], in_=pooledP[34:68, 512:640]
                    )

                conv1_cfg = [
                    ("pe1", OFF_CE1, 128),
                    ("po1", OFF_CO1, 128),
                    ("pem", OFF_CEM, 128),
                    ("pom", OFF_COM, 128),
                    ("pe2", OFF_CE2, 112),
                    ("po2", OFF_CO2, 112),
                ]
                hps_tag = "hps" if TUNE.get("hps_bufs", 0) else "ps"

                def _head_chunk(c0, C):
                    rhs1 = pooledU[:, c0 : c0 + C]
                    cps = {}
                    for nm, off, m in conv1_cfg:
                        p = hps.tile([128, C], f32, name=nm, tag=hps_tag)
                        nc.tensor.matmul(
                            p[0:m, :],
                            cstt[0:34, off : off + m],
                            rhs1,
                            start=True,
                            stop=True,
                        )
                        cps[nm] = p
                    # maxpool pairs; ReLU+bias applied after the max (commute)
                    c1b = cstf_t[0:128, FB_C1B : FB_C1B + 1]
                    mts = {}
                    for nm, a, b, m in [
                        ("ma", "pe1", "po1", 128),
                        ("mb", "pem", "pom", 128),
                        ("mc", "pe2", "po2", 112),
                    ]:
                        # only one PSUM operand per DVE op: relu one side on
                        # ACT, fuse (+bias, max) for the other on DVE.
                        se = hsb.tile([128, C], f32r, name=f"{nm}_se")
                        nc.scalar.activation(
                            se[0:m, :], cps[a][0:m, :], Relu, bias=c1b[0:m, :]
                        )
                        mt = hsb.tile([128, C], f32r, name=nm)
                        nc.vector.scalar_tensor_tensor(
                            mt[0:m, :], cps[b][0:m, :], c1b[0:m, :],
                            se[0:m, :], ADD, MAX,
                        )
                        mts[nm] = mt
                    # conv2 (banded matmuls over the pooled u-rows)
                    c2b = cstf_t[0:128, FB_C2B : FB_C2B + 1]
                    rs = []
                    for nm, off, srcn, kk, m in [
                        ("pc0", OFF_C20, "ma", 128, 128),
                        ("pc1", OFF_C21, "mb", 128, 128),
                        ("pc2", OFF_C22, "mc", 112, 96),
                    ]:
                        p = hps.tile([128, C], f32, name=nm, tag=hps_tag)
                        nc.tensor.matmul(
                            p[0:m, :],
                            cstt[0:kk, off : off + m],
                            mts[srcn][0:kk, :],
                            start=True,
                            stop=True,
                        )
                        r = hsb.tile([128, C], f32r, name=f"r{nm}")
                        if TUNE.get("head_c2relu", "act") == "act":
                            nc.scalar.activation(
                                r[0:m, :], p[0:m, :], Relu, bias=c2b[0:m, :]
                            )
                        else:
                            nc.vector.tensor_scalar(
                                r[0:m, :], p[0:m, :], c2b[0:m, :], 0.0, ADD, MAX
                            )
                        rs.append((r, m))
                    # final linear [352 -> 2], accumulated over the row groups
                    py = hps.tile([2, C], f32, name="py", tag=hps_tag)
                    for gi, (off, (r, m)) in enumerate(
                        zip([OFF_WO0, OFF_WO1, OFF_WO2], rs)
                    ):
                        nc.tensor.matmul(
                            py[:, :],
                            cstt[0:m, off : off + 2],
                            r[0:m, :],
                            start=(gi == 0),
                            stop=(gi == 2),
                        )
                    nc.scalar.add(
                        ysb[:, c0 : c0 + C],
                        py[:, :],
                        add=cstf_t[0:2, FB_BO : FB_BO + 1],
                    )
                    y_eng = nc.gpsimd if TUNE.get("ydma_swdge") else nc.sync
                    y_eng.dma_start(
                        out=y[:, c0 : c0 + C], in_=ysb[:, c0 : c0 + C]
                    )

                def _mlp_tile(i, _rep):
                    tile_emap = emap
                    alt = TUNE.get("emap_alt")
                    if alt and i % 2 == 1:
                        tile_emap = alt
                    if _rep == 0 and i == 0:
                        # tile 0 fills the pipeline; a DVE-leaning L1 map
                        # starts both relu engines earlier
                        tile_emap = TUNE.get("emap0", "daddadadadadaadd")
                    if i == N_TILES - 1:
                        # tile 9's chunk 3 is the tiny ragged chunk; a
                        # 6-DVE map balances better there
                        tile_emap = TUNE.get("emap_last", "aaaaadadadaadadd")
                    if _rep == 0 and i in xt_pre:
                        xt_i = xt_pre.pop(i)
                    else:
                        xt_i = xp.tile([128, TILE_COLS], f32r, name="xt_i")
                        nsp = TUNE.get("xdma_split", 1)
                        step = TILE_COLS // nsp
                        for s in range(nsp):
                            nc.sync.dma_start(
                                out=xt_i[:, s * step : (s + 1) * step],
                                in_=xt[i, :, s * step : (s + 1) * step],
                            )
                    cur = xt_i
                    pair = TUNE.get("pair", "")  # '', 'dve', 'both'
                    # last tile: chunk 3 holds a single real graph (no. 624);
                    # the other 15 slots are host-padding nobody reads.
                    rag = i == N_TILES - 1 and TUNE.get("ragged_tail", True)

                    def _dc(c):
                        return K if (rag and c == N_CHUNKS - 1) else DATA_COLS
                    for li, (woff, wm, boff, outp) in enumerate(layer_cfg):
                        b_ap = cstf_t[0:outp, boff : boff + 1]
                        w_ap = cstt[:, woff : woff + wm]
                        h = hp.tile([128, TILE_COLS], f32r, name=f"h{li}")
                        groups = []
                        c = 0
                        while c < N_CHUNKS:
                            e = tile_emap[li * N_CHUNKS + c]
                            pairable = (
                                c % 2 == 0
                                and c + 1 < N_CHUNKS
                                and tile_emap[li * N_CHUNKS + c + 1] == e
                                and (pair == "both" or (pair == "dve" and e == "d"))
                                and not (rag and c + 1 == N_CHUNKS - 1)
                            )
                            if pairable:
                                groups.append((c, 2))
                                c += 2
                            else:
                                groups.append((c, 1))
                                c += 1
                        for c0, gran in groups:
                            pool_ = mps2 if gran == 2 else mps
                            ps = pool_.tile(
                                [128, gran * CHUNK_COLS], f32, name="ps",
                                tag="ps2" if gran == 2 else "ps",
                            )
                            dcc = _dc(c0 + gran - 1)
                            for g in range(gran):
                                c = c0 + g
                                nc.tensor.matmul(
                                    ps[
                                        0:outp,
                                        g * CHUNK_COLS : g * CHUNK_COLS
                                        + _dc(c),
                                    ],
                                    w_ap,
                                    cur[
                                        :,
                                        c * CHUNK_COLS : c * CHUNK_COLS
                                        + _dc(c),
                                    ],
                                    start=True,
                                    stop=True,
                                )
                            ps_v = ps[0:outp, :].rearrange(
                                "p (c s) -> p c s", c=gran
                            )[:, :, 0:dcc]
                            h_v = h[
                                0:outp,
                                c0 * CHUNK_COLS : (c0 + gran) * CHUNK_COLS,
                            ].rearrange("p (c s) -> p c s", c=gran)[
                                :, :, 0:dcc
                            ]
                            hs = TUNE.get("half_split")
                            if hs and gran == 1 and (li, c0) == hs[0] \
                                    and _dc(c0) == DATA_COLS:
                                # engine-split chunk: fine-grained ACT/DVE
                                # balancing (hs = ((layer, chunk), act_cols))
                                acols = hs[1]
                                nc.scalar.activation(
                                    h_v[:, :, 0:acols],
                                    ps_v[:, :, 0:acols], Relu, bias=b_ap,
                                )
                                nc.vector.tensor_scalar(
                                    h_v[:, :, acols:DATA_COLS],
                                    ps_v[:, :, acols:DATA_COLS],
                                    b_ap, 0.0, ADD, MAX,
                                )
                            elif tile_emap[li * N_CHUNKS + c0] == "a":
                                nc.scalar.activation(h_v, ps_v, Relu, bias=b_ap)
                            else:
                                nc.vector.tensor_scalar(
                                    h_v, ps_v, b_ap, 0.0, ADD, MAX
                                )
                        cur = h
                    # mean-pool (sum; /30 folded into conv1 weights):
                    # per-chunk GPSIMD halve (30->15) + DVE reduce.
                    for c in range(N_CHUNKS):
                        dcc = _dc(c)
                        h4v = cur[
                            0:68, c * CHUNK_COLS : c * CHUNK_COLS + dcc
                        ].rearrange("p (g k) -> p g k", k=K)
                        outv = pooledP[
                            :,
                            i * TILE_G + c * 16 : i * TILE_G + c * 16 + dcc // K,
                        ]
                        with nc.allow_low_precision(
                            reason="float32r is 4-byte fp32 storage"
                        ):
                            if TUNE["pool_mode"] == "gpsimd_halve":
                                hh = hp.tile(
                                    [68, DATA_COLS // 2], f32r, name="hh"
                                )
                                hhv = hh[:, 0 : dcc // 2].rearrange(
                                    "p (g k) -> p g k", k=K // 2
                                )
                                nc.gpsimd.tensor_tensor(
                                    hhv, h4v[:, :, 0 : K // 2],
                                    h4v[:, :, K // 2 : K], op=ADD,
                                )
                                nc.vector.tensor_reduce(
                                    outv, hhv, axis=AX, op=ADD
                                )
                            else:
                                nc.vector.tensor_reduce(
                                    outv, h4v, axis=AX, op=ADD
                                )

                n_rep = TUNE.get("repeat", 1)
                head_il = TUNE.get("head_interleave", True) and not TUNE["skip_head"]
                for _rep in range(n_rep):
                    last_rep = _rep == n_rep - 1
                    for i in range(N_TILES):
                        _mlp_tile(i, _rep)
                        if head_il and last_rep:
                            if i == 7:
                                _unpack_early()
                            elif i == 8:
                                _head_chunk(0, 512)
                                _head_chunk(512, 512)
                if TUNE["skip_head"]:
                    nc.sync.dma_start(
                        out=y[:, :],
                        in_=pooledP[0:2, 0:HEAD_COLS].bitcast(f32),
                    )
                    return nc
                if head_il:
                    _unpack_late()
                    _head_chunk(1024, 256)
                else:
                    _unpack_early()
                    _unpack_late()
                    for c0, C in TUNE.get(
                        "head_chunks", [(0, 512), (512, 512), (1024, 256)]
                    ):
                        _head_chunk(c0, C)
    return nc


def _get_nc():
    if "nc" not in _NC_CACHE:
        _NC_CACHE["nc"] = _build_nc()
    return _NC_CACHE["nc"]


# ------------------------------------------------------------------ host prep
def _prep_x(x):
    """[N, 64] -> per-core [N_TILES, 128, 2048] packed transposed tiles."""
    xs = np.ascontiguousarray(x.reshape(G, NODES_PER_G, F)[:, :K, :])
    out = np.zeros((NCORE, N_TILES, 128, TILE_COLS), np.float32)
    for c in range(NCORE):
        for half in range(2):
            gs = c * G_CORE + half * G_HALF
            segp = np.zeros((G_HALF_PAD, K, F), np.float32)
            segp[:G_HALF] = xs[gs : gs + G_HALF]
            a = segp.reshape(N_TILES, N_CHUNKS, 16, K, F)
            a = np.ascontiguousarray(a.transpose(0, 4, 1, 2, 3)).reshape(
                N_TILES, F, N_CHUNKS, DATA_COLS
            )
            dst = out[c][:, half * F : (half + 1) * F, :].reshape(
                N_TILES, F, N_CHUNKS, CHUNK_COLS
            )
            dst[..., :DATA_COLS] = a
    return out


def _build_const(W1, b1, W2, b2, W3, b3, W4, b4, cw1, cb1, cw2, cb2, Wo, bo):
    cst = np.zeros((128, NCOL_PAD), np.float32)

    def bd(W):  # torch [out, in] -> block-diag lhsT [128, 2*out]
        o = W.shape[0]
        m = np.zeros((128, 2 * o), np.float32)
        m[0:64, 0:o] = W.T
        m[64:128, o : 2 * o] = W.T
        return m

    cst[:, OFF_W1 : OFF_W1 + 128] = bd(W1)
    cst[:, OFF_W2 : OFF_W2 + 128] = bd(W2)
    cst[:, OFF_W3 : OFF_W3 + 128] = bd(W3)
    cst[:, OFF_W4 : OFF_W4 + 68] = bd(W4)

    def conv1_lhsT(ts):  # [34, 16*len(ts)]; includes the 1/30 mean fold
        m = np.zeros((34, 16 * len(ts)), np.float32)
        for ul, t in enumerate(ts):
            for oc in range(16):
                m[t : t + 5, ul * 16 + oc] = cw1[oc, 0, :] / float(K)
        return m

    cst[0:34, OFF_CE1 : OFF_CE1 + 128] = conv1_lhsT(range(0, 16, 2))
    cst[0:34, OFF_CO1 : OFF_CO1 + 128] = conv1_lhsT(range(1, 16, 2))
    cst[0:34, OFF_CEM : OFF_CEM + 128] = conv1_lhsT(range(8, 24, 2))
    cst[0:34, OFF_COM : OFF_COM + 128] = conv1_lhsT(range(9, 24, 2))
    cst[0:34, OFF_CE2 : OFF_CE2 + 112] = conv1_lhsT(range(16, 30, 2))
    cst[0:34, OFF_CO2 : OFF_CO2 + 112] = conv1_lhsT(range(17, 30, 2))

    def conv2_lhsT(tgs, us):  # [16*len(us), 32*len(tgs)]
        m = np.zeros((16 * len(us), 32 * len(tgs)), np.float32)
        for ri, u in enumerate(us):
            for ci, t in enumerate(tgs):
                kk = u - t
                if 0 <= kk < 5:
                    for ic in range(16):
                        m[ri * 16 + ic, ci * 32 : (ci + 1) * 32] = cw2[:, ic, kk]
        return m

    cst[0:128, OFF_C20 : OFF_C20 + 128] = conv2_lhsT(range(0, 4), range(0, 8))
    cst[0:128, OFF_C21 : OFF_C21 + 128] = conv2_lhsT(range(4, 8), range(4, 12))
    cst[0:112, OFF_C22 : OFF_C22 + 96] = conv2_lhsT(range(8, 11), range(8, 15))

    def wo_map(ts):  # [32*len(ts), 2]; undo the oc2-major flatten order
        m = np.zeros((32 * len(ts), 2), np.float32)
        for ci, t in enumerate(ts):
            for oc2 in range(32):
                m[ci * 32 + oc2, :] = Wo[:, oc2 * 11 + t]
        return m

    cst[0:128, OFF_WO0 : OFF_WO0 + 2] = wo_map(range(0, 4))
    cst[0:128, OFF_WO1 : OFF_WO1 + 2] = wo_map(range(4, 8))
    cst[0:96, OFF_WO2 : OFF_WO2 + 2] = wo_map(range(8, 11))

    cstf = np.zeros((128, 8), np.float32)
    cstf[0:128, FB_B1] = np.concatenate([b1, b1])
    cstf[0:128, FB_B2] = np.concatenate([b2, b2])
    cstf[0:128, FB_B3] = np.concatenate([b3, b3])
    cstf[0:68, FB_B4] = np.concatenate([b4, b4])
    cstf[0:128, FB_C1B] = np.tile(cb1, 8)
    cstf[0:128, FB_C2B] = np.tile(cb2, 4)
    cstf[0:2, FB_BO] = bo
    return cst, cstf


def _numpy_forward(x, batch, W1, b1, W2, b2, W3, b3, W4, b4, cw1, cb1, cw2, cb2, Wo, bo):
    """General (slow) host fallback, used only if batch is not arange//100."""
    h = np.maximum(x @ W1.T + b1, 0)
    h = np.maximum(h @ W2.T + b2, 0)
    h = np.maximum(h @ W3.T + b3, 0)
    h = np.maximum(h @ W4.T + b4, 0)
    counts = np.bincount(batch, minlength=G).astype(np.float32)
    starts = np.cumsum(counts) - counts
    pos = np.arange(h.shape[0], dtype=np.float32) - starts[batch]
    mask = (pos < K).astype(np.float32)
    sums = np.zeros((G, h.shape[1]), np.float32)
    np.add.at(sums, batch, h * mask[:, None])
    denom = np.minimum(counts, float(K))
    pooled = sums / denom[:, None]
    c1 = np.zeros((G, 16, 30), np.float32)
    for t in range(30):
        c1[:, :, t] = pooled[:, t : t + 5] @ cw1[:, 0, :].T
    c1 = np.maximum(c1 + cb1[None, :, None], 0)
    m = np.maximum(c1[:, :, 0::2], c1[:, :, 1::2])  # [G, 16, 15]
    c2 = np.zeros((G, 32, 11), np.float32)
    for t in range(11):
        c2[:, :, t] = np.einsum("gik,oik->go", m[:, :, t : t + 5], cw2)
    c2 = np.maximum(c2 + cb2[None, :, None], 0)
    flat = c2.reshape(G, -1)
    return flat @ Wo.T + bo


def _run(inputs, trace=False, trace_kwargs=None):
    """Returns (y [10000, 2], BassKernelResults-or-None)."""
    x = np.ascontiguousarray(np.asarray(inputs["x"], dtype=np.float32))
    batch = np.asarray(inputs["batch"])
    names = ["W1", "b1", "W2", "b2", "W3", "b3", "W4", "b4",
             "cw1", "cb1", "cw2", "cb2", "Wo", "bo"]
    ws = [np.ascontiguousarray(np.asarray(inputs[n], dtype=np.float32)) for n in names]

    expected_batch = (np.arange(N, dtype=np.int64) // (N // G)).astype(batch.dtype)
    if batch.shape != (N,) or not np.array_equal(batch, expected_batch):
        return _numpy_forward(x, np.asarray(batch, np.int64), *ws), None

    from concourse.bass_utils import run_bass_kernel_spmd

    nc = _get_nc()
    xt_all = _prep_x(x)
    cst, cstf = _build_const(*ws)
    in_maps = [{"xt": xt_all[c], "cst": cst, "cstf": cstf} for c in range(NCORE)]
    kw = {}
    if trace:
        kw["trace"] = True
        if trace_kwargs:
            kw["trace_kwargs"] = trace_kwargs
    res = run_bass_kernel_spmd(nc, in_maps, core_ids=list(range(NCORE)), **kw)

    out = np.empty((G, 2), np.float32)
    for c in range(NCORE):
        yc = res.results[c]["y"]
        base = c * G_CORE
        # pooledU column order (see _build_nc): A 0:512 | B 0:512 |
        # A 512:640 | B 512:640 ; valid graphs are 0:625 per half.
        out[base : base + 512] = yc[:, 0:512].T
        out[base + 512 : base + G_HALF] = yc[:, 1024 : 1024 + G_HALF - 512].T
        out[base + G_HALF : base + G_HALF + 512] = yc[:, 512:1024].T
        out[base + G_HALF + 512 : base + G_CORE] = yc[
            :, 1152 : 1152 + G_HALF - 512
        ].T
    return out, res


def kernel(**inputs) -> np.ndarray:
    out, _ = _run(inputs)
    return out

